# revision 4
# baseline (speedup 1.0000x reference)
"""GNN message-passing (gather + segment_sum) Trainium2 Bass kernel.

Strategy (dst-sharded, SBUF-resident quartered feature table, zero
per-edge DMA):
  - Core c owns dst nodes [c*6272, (c+1)*6272) (49 groups of 128); host
    sorts edges by (core, dst-group, src-quarter) and packs int16 gather
    indices + per-slot group-relative dst values.
  - The x table lives in SBUF transposed ([feat, node]) with partition
    p = 32*lane + feat; lane i holds nodes [i*12544, (i+1)*12544), so
    the table is stored once (6.4MB, one linear DMA load - no per-edge
    descriptors, which cost ~1us each on HW and dominated the previous
    dma_gather kernel). Edges are routed to the lane owning their src.
  - Each 16-partition GpSimd core gathers feature planes for its lane's
    edge list via ap_gather (SBUF->SBUF ucode gather along the free dim,
    local node index < 12544 fits int16).
  - DVE StreamTranspose (32x32 blocks) of the [128, CALL] gather output
    directly yields [128 edge, 32 feat] chunks thanks to the 32i+f
    partition layout: chunk j = 32 edges per lane at positions
    [32j, 32j+32).
  - Per 128-edge chunk: one-hot B[edge, dst_rel] built on DVE (batched
    via broadcast APs), TensorE accumulates B^T @ msgs into a per-group
    PSUM partial, copied into an SBUF accumulator; one DMA writes the
    core's [128, 49, 32] output.

Self-contained: hardcodes the problem shapes from the spec.
"""

import numpy as np

import concourse.bass as bass
import concourse.tile as tile
from concourse import bacc, mybir
from concourse.alu_op_type import AluOpType
from concourse.bass_utils import run_bass_kernel_spmd

N_NODES = 50000
D_FEAT = 32
N_CORES = 8
G = 128  # dst nodes per group
GROUPS_PER_CORE = 49
QN = 12544  # nodes per lane quarter (4 * 12544 = 50176 >= 50000)
CALL = 4096  # ap_gather num_idxs per call (per 16-partition core)
CPC = CALL // 32  # chunks of 128 edges per call
BSEL_CH = 16  # chunks per one-hot build instruction


def _prep(x, edge_index):
    """Host-side packing. Returns per-core input maps + schedule constants."""
    src = np.asarray(edge_index[0], dtype=np.int64)
    dst = np.asarray(edge_index[1], dtype=np.int64)
    E = src.shape[0]

    ag = dst >> 7
    core = ag // GROUPS_PER_CORE
    g = ag % GROUPS_PER_CORE
    qtr = src // QN
    loc = src % QN

    okey = (core * GROUPS_PER_CORE + g) * 4 + qtr
    cnt = np.bincount(okey, minlength=N_CORES * GROUPS_PER_CORE * 4)
    cnt3 = cnt.reshape(N_CORES, GROUPS_PER_CORE, 4)
    # chunk count per group: max over cores and quarters so the SPMD program
    # is identical and every lane's 32-slot block fits.
    C = np.maximum(1, -(-cnt3.max(axis=(0, 2)) // 32))  # [49]
    chunks_tot = int(C.sum())
    n_calls = -(-chunks_tot // CPC)
    off = np.zeros(GROUPS_PER_CORE, np.int64)
    off[1:] = np.cumsum(C)[:-1]

    idx_cols = n_calls * (CALL // 16)
    dr_cols = n_calls * CPC

    order = np.argsort(okey, kind="stable")
    starts = np.concatenate([[0], np.cumsum(cnt)[:-1]])
    pos = np.arange(E) - starts[okey[order]]

    core_s = core[order]
    g_s = g[order]
    qtr_s = qtr[order]
    loc_s = loc[order]
    dst_s = dst[order]

    l = pos >> 5
    a = pos & 31
    k = off[g_s] + l
    m = k // CPC
    j = k % CPC
    t = 32 * j + a
    col = m * (CALL // 16) + (t >> 4)
    row = 32 * qtr_s + (t & 15)

    idx_arr = np.zeros((N_CORES, 128, idx_cols), np.int16)
    idxval = loc_s.astype(np.int16)
    idx_arr[core_s, row, col] = idxval
    idx_arr[core_s, row + 16, col] = idxval

    dr_arr = np.full((N_CORES, 128, dr_cols), -1.0, np.float32)
    q = 32 * qtr_s + a
    dr_arr[core_s, q, m * CPC + j] = (dst_s & 127).astype(np.float32)

    xT = np.zeros((D_FEAT, 4 * QN), np.float32)
    xT[:, :N_NODES] = np.asarray(x, dtype=np.float32).T
    iota = np.broadcast_to(
        np.arange(G, dtype=np.float32)[None, :], (128, G)
    ).copy()

    ins = []
    for c in range(N_CORES):
        ins.append(
            {
                "xT": xT,
                "idx16": idx_arr[c],
                "dstrel": dr_arr[c],
                "iota": iota,
            }
        )
    sched = (tuple(int(v) for v in C), n_calls)
    return ins, sched, idx_cols, dr_cols


def _build(reps, sched, idx_cols, dr_cols):
    C, n_calls = sched
    chunks_tot = sum(C)
    nc = bacc.Bacc(
        "TRN2", target_bir_lowering=False, debug=False, num_devices=N_CORES
    )
    f32 = mybir.dt.float32
    xT = nc.dram_tensor("xT", [D_FEAT, 4 * QN], f32, kind="ExternalInput")
    idx16 = nc.dram_tensor(
        "idx16", [128, idx_cols], mybir.dt.int16, kind="ExternalInput"
    )
    dstrel = nc.dram_tensor("dstrel", [128, dr_cols], f32, kind="ExternalInput")
    iota = nc.dram_tensor("iota", [128, G], f32, kind="ExternalInput")
    out = nc.dram_tensor(
        "out", [128, GROUPS_PER_CORE, D_FEAT], f32, kind="ExternalOutput"
    )

    # chunk -> (group, local chunk index)
    chunk_map = []
    for gg in range(GROUPS_PER_CORE):
        for ll in range(C[gg]):
            chunk_map.append((gg, ll))

    with tile.TileContext(nc) as tc:
        with (
            tc.tile_pool(name="meta", bufs=1) as meta,
            tc.tile_pool(name="tab", bufs=1) as tabp,
            tc.tile_pool(name="msgT", bufs=3) as msgTp,
            tc.tile_pool(name="msgt", bufs=3) as msgtp,
            tc.tile_pool(name="bsel", bufs=4) as bselp,
            tc.tile_pool(name="ps", bufs=4, space="PSUM") as psp,
            tc.tile_pool(name="osb", bufs=1) as osbp,
        ):
            idx_t = meta.tile([128, idx_cols], mybir.dt.int16)
            nc.sync.dma_start(idx_t[:], idx16.ap())
            dr_t = meta.tile([128, dr_cols], f32)
            nc.sync.dma_start(dr_t[:], dstrel.ap())
            iota_t = meta.tile([128, G], f32)
            nc.sync.dma_start(iota_t[:], iota.ap())

            def body(_=None):
                out_sb = osbp.tile([128, GROUPS_PER_CORE * D_FEAT], f32)
                tab = tabp.tile([128, QN], f32)
                for i in range(4):
                    nc.sync.dma_start(
                        tab[32 * i : 32 * i + 32, :],
                        xT.ap()[:, i * QN : (i + 1) * QN],
                    )
                ps_open = None
                for mm in range(n_calls):
                    msgs_T = msgTp.tile([128, CALL], f32)
                    nc.gpsimd.ap_gather(
                        msgs_T[:],
                        tab[:],
                        idx_t[:, mm * (CALL // 16) : (mm + 1) * (CALL // 16)],
                        128,
                        QN,
                        1,
                        CALL,
                    )
                    msgs_t = msgtp.tile([128, CALL], f32)
                    nc.vector.transpose(msgs_t[:], msgs_T[:])
                    for hb in range(CPC // BSEL_CH):
                        bt = bselp.tile([128, BSEL_CH, G], f32)
                        c0 = mm * CPC + hb * BSEL_CH
                        dr_b = (
                            dr_t[:, c0 : c0 + BSEL_CH]
                            .unsqueeze(2)
                            .broadcast_to((128, BSEL_CH, G))
                        )
                        iota_b = (
                            iota_t[:]
                            .unsqueeze(1)
                            .broadcast_to((128, BSEL_CH, G))
                        )
                        nc.vector.scalar_tensor_tensor(
                            bt[:],
                            iota_b,
                            0.0,
                            dr_b,
                            AluOpType.add,
                            AluOpType.is_equal,
                        )
                        for cc in range(BSEL_CH):
                            k = mm * CPC + hb * BSEL_CH + cc
                            if k >= chunks_tot:
                                continue
                            gg, ll = chunk_map[k]
                            if ll == 0:
                                ps_open = psp.tile([128, D_FEAT], f32)
                            e0 = 32 * (hb * BSEL_CH + cc)
                            nc.tensor.matmul(
                                out=ps_open[:],
                                lhsT=bt[:, cc, :],
                                rhs=msgs_t[:, e0 : e0 + D_FEAT],
                                start=(ll == 0),
                                stop=(ll == C[gg] - 1),
                            )
                            if ll == C[gg] - 1:
                                nc.scalar.copy(
                                    out_sb[:, gg * D_FEAT : (gg + 1) * D_FEAT],
                                    ps_open[:],
                                )
                nc.sync.dma_start(out.ap(), out_sb[:])

            if reps == 1:
                body()
            else:
                with tc.For_i(0, reps) as _i:
                    body(_i)
    nc.compile()
    return nc


_CACHE = {}


def _get_nc(reps, sched, idx_cols, dr_cols):
    key = (reps, sched, idx_cols, dr_cols)
    if key not in _CACHE:
        _CACHE[key] = _build(reps, sched, idx_cols, dr_cols)
    return _CACHE[key]


_PREP_CACHE = {}


def _prep_cached(x, edge_index):
    key = (id(x), id(edge_index))
    if key not in _PREP_CACHE:
        _PREP_CACHE.clear()
        _PREP_CACHE[key] = _prep(x, edge_index)
    return _PREP_CACHE[key]


def run(x, edge_index, reps=1):
    ins, sched, idx_cols, dr_cols = _prep_cached(x, edge_index)
    nc = _get_nc(reps, sched, idx_cols, dr_cols)
    res = run_bass_kernel_spmd(nc, ins, core_ids=list(range(N_CORES)))
    parts = []
    for c in range(N_CORES):
        o = res.results[c]["out"]  # [128, 49, 32]
        parts.append(np.transpose(o, (1, 0, 2)).reshape(-1, D_FEAT))
    return np.concatenate(parts, axis=0)[:N_NODES]


def kernel(x, edge_index):
    return run(x, edge_index, reps=1)


# revision 5
# speedup vs baseline: 1.1961x; 1.1961x over previous
"""GNN message-passing (gather + segment_sum) Trainium2 Bass kernel.

Strategy (dst-sharded, SBUF-resident quartered feature table, zero
per-edge DMA):
  - Core c owns dst nodes [c*6272, (c+1)*6272) (49 groups of 128); host
    sorts edges by (core, dst-group, src-quarter) and packs int16 gather
    indices + per-slot group-relative dst values.
  - The x table lives in SBUF transposed ([feat, node]) with partition
    p = 32*lane + feat; lane i holds nodes [i*12544, (i+1)*12544), so
    the table is stored once (6.4MB, one linear DMA load - no per-edge
    descriptors, which cost ~1us each on HW and dominated the previous
    dma_gather kernel). Edges are routed to the lane owning their src.
  - Each 16-partition GpSimd core gathers feature planes for its lane's
    edge list via ap_gather (SBUF->SBUF ucode gather along the free dim,
    local node index < 12544 fits int16).
  - DVE StreamTranspose (32x32 blocks) of the [128, CALL] gather output
    directly yields [128 edge, 32 feat] chunks thanks to the 32i+f
    partition layout: chunk j = 32 edges per lane at positions
    [32j, 32j+32).
  - Per 128-edge chunk: one-hot B[edge, dst_rel] built on DVE (batched
    via broadcast APs), TensorE accumulates B^T @ msgs into a per-group
    PSUM partial, copied into an SBUF accumulator; one DMA writes the
    core's [128, 49, 32] output.

Self-contained: hardcodes the problem shapes from the spec.
"""

import numpy as np

import concourse.bass as bass
import concourse.tile as tile
from concourse import bacc, mybir
from concourse.alu_op_type import AluOpType
from concourse.bass_utils import run_bass_kernel_spmd

N_NODES = 50000
D_FEAT = 32
N_CORES = 8
G = 128  # dst nodes per group
GROUPS_PER_CORE = 49
QN = 12544  # nodes per lane quarter (4 * 12544 = 50176 >= 50000)
CALL = 2048  # ap_gather num_idxs per call (per 16-partition core)
CPC = CALL // 32  # chunks of 128 edges per call
BSEL_CH = 32  # chunks per one-hot build instruction


def _prep(x, edge_index):
    """Host-side packing. Returns per-core input maps + schedule constants."""
    src = np.asarray(edge_index[0], dtype=np.int64)
    dst = np.asarray(edge_index[1], dtype=np.int64)
    E = src.shape[0]

    ag = dst >> 7
    core = ag // GROUPS_PER_CORE
    g = ag % GROUPS_PER_CORE
    qtr = src // QN
    loc = src % QN

    okey = (core * GROUPS_PER_CORE + g) * 4 + qtr
    cnt = np.bincount(okey, minlength=N_CORES * GROUPS_PER_CORE * 4)
    cnt3 = cnt.reshape(N_CORES, GROUPS_PER_CORE, 4)
    # chunk count per group: max over cores and quarters so the SPMD program
    # is identical and every lane's 32-slot block fits.
    C = np.maximum(1, -(-cnt3.max(axis=(0, 2)) // 32))  # [49]
    chunks_tot = int(C.sum())
    n_calls = -(-chunks_tot // CPC)
    off = np.zeros(GROUPS_PER_CORE, np.int64)
    off[1:] = np.cumsum(C)[:-1]

    idx_cols = n_calls * (CALL // 16)
    dr_cols = n_calls * CPC

    order = np.argsort(okey, kind="stable")
    starts = np.concatenate([[0], np.cumsum(cnt)[:-1]])
    pos = np.arange(E) - starts[okey[order]]

    core_s = core[order]
    g_s = g[order]
    qtr_s = qtr[order]
    loc_s = loc[order]
    dst_s = dst[order]

    l = pos >> 5
    a = pos & 31
    k = off[g_s] + l
    m = k // CPC
    j = k % CPC
    t = 32 * j + a
    col = m * (CALL // 16) + (t >> 4)
    row = 32 * qtr_s + (t & 15)

    idx_arr = np.zeros((N_CORES, 128, idx_cols), np.int16)
    idxval = loc_s.astype(np.int16)
    idx_arr[core_s, row, col] = idxval
    idx_arr[core_s, row + 16, col] = idxval

    dr_arr = np.full((N_CORES, 128, dr_cols), -1.0, np.float32)
    q = 32 * qtr_s + a
    dr_arr[core_s, q, m * CPC + j] = (dst_s & 127).astype(np.float32)

    xT = np.zeros((D_FEAT, 4 * QN), np.float32)
    xT[:, :N_NODES] = np.asarray(x, dtype=np.float32).T
    iota = np.broadcast_to(
        np.arange(G, dtype=np.float32)[None, :], (128, G)
    ).copy()

    ins = []
    for c in range(N_CORES):
        ins.append(
            {
                "xT": xT,
                "idx16": idx_arr[c],
                "dstrel": dr_arr[c],
                "iota": iota,
            }
        )
    sched = (tuple(int(v) for v in C), n_calls)
    return ins, sched, idx_cols, dr_cols


def _build(reps, sched, idx_cols, dr_cols):
    C, n_calls = sched
    chunks_tot = sum(C)
    nc = bacc.Bacc(
        "TRN2", target_bir_lowering=False, debug=False, num_devices=N_CORES
    )
    f32 = mybir.dt.float32
    xT = nc.dram_tensor("xT", [D_FEAT, 4 * QN], f32, kind="ExternalInput")
    idx16 = nc.dram_tensor(
        "idx16", [128, idx_cols], mybir.dt.int16, kind="ExternalInput"
    )
    dstrel = nc.dram_tensor("dstrel", [128, dr_cols], f32, kind="ExternalInput")
    iota = nc.dram_tensor("iota", [128, G], f32, kind="ExternalInput")
    out = nc.dram_tensor(
        "out", [128, GROUPS_PER_CORE, D_FEAT], f32, kind="ExternalOutput"
    )

    # chunk -> (group, local chunk index)
    chunk_map = []
    for gg in range(GROUPS_PER_CORE):
        for ll in range(C[gg]):
            chunk_map.append((gg, ll))

    with tile.TileContext(nc) as tc:
        with (
            tc.tile_pool(name="meta", bufs=1) as meta,
            tc.tile_pool(name="tab", bufs=1) as tabp,
            tc.tile_pool(name="msgT", bufs=3) as msgTp,
            tc.tile_pool(name="msgt", bufs=3) as msgtp,
            tc.tile_pool(name="bsel", bufs=3) as bselp,
            tc.tile_pool(name="ps", bufs=4, space="PSUM") as psp,
            tc.tile_pool(name="osb", bufs=1) as osbp,
        ):
            idx_t = meta.tile([128, idx_cols], mybir.dt.int16)
            nc.sync.dma_start(idx_t[:], idx16.ap())
            dr_t = meta.tile([128, dr_cols], f32)
            nc.sync.dma_start(dr_t[:], dstrel.ap())
            iota_t = meta.tile([128, G], f32)
            nc.sync.dma_start(iota_t[:], iota.ap())

            def body(_=None):
                out_sb = osbp.tile([128, GROUPS_PER_CORE * D_FEAT], f32)
                tab = tabp.tile([128, QN], f32)
                for i in range(4):
                    nc.sync.dma_start(
                        tab[32 * i : 32 * i + 32, :],
                        xT.ap()[:, i * QN : (i + 1) * QN],
                    )
                ps_open = None
                for mm in range(n_calls):
                    msgs_T = msgTp.tile([128, CALL], f32)
                    nc.gpsimd.ap_gather(
                        msgs_T[:],
                        tab[:],
                        idx_t[:, mm * (CALL // 16) : (mm + 1) * (CALL // 16)],
                        128,
                        QN,
                        1,
                        CALL,
                    )
                    msgs_t = msgtp.tile([128, CALL], f32)
                    nc.vector.transpose(msgs_t[:], msgs_T[:])
                    for hb in range(CPC // BSEL_CH):
                        bt = bselp.tile([128, BSEL_CH, G], f32)
                        c0 = mm * CPC + hb * BSEL_CH
                        dr_b = (
                            dr_t[:, c0 : c0 + BSEL_CH]
                            .unsqueeze(2)
                            .broadcast_to((128, BSEL_CH, G))
                        )
                        iota_b = (
                            iota_t[:]
                            .unsqueeze(1)
                            .broadcast_to((128, BSEL_CH, G))
                        )
                        nc.vector.scalar_tensor_tensor(
                            bt[:],
                            iota_b,
                            0.0,
                            dr_b,
                            AluOpType.add,
                            AluOpType.is_equal,
                        )
                        for cc in range(BSEL_CH):
                            k = mm * CPC + hb * BSEL_CH + cc
                            if k >= chunks_tot:
                                continue
                            gg, ll = chunk_map[k]
                            if ll == 0:
                                ps_open = psp.tile([128, D_FEAT], f32)
                            e0 = 32 * (hb * BSEL_CH + cc)
                            nc.tensor.matmul(
                                out=ps_open[:],
                                lhsT=bt[:, cc, :],
                                rhs=msgs_t[:, e0 : e0 + D_FEAT],
                                start=(ll == 0),
                                stop=(ll == C[gg] - 1),
                            )
                            if ll == C[gg] - 1:
                                nc.scalar.copy(
                                    out_sb[:, gg * D_FEAT : (gg + 1) * D_FEAT],
                                    ps_open[:],
                                )
                nc.sync.dma_start(out.ap(), out_sb[:])

            if reps == 1:
                body()
            else:
                with tc.For_i(0, reps) as _i:
                    body(_i)
    nc.compile()
    return nc


_CACHE = {}


def _get_nc(reps, sched, idx_cols, dr_cols):
    key = (reps, sched, idx_cols, dr_cols)
    if key not in _CACHE:
        _CACHE[key] = _build(reps, sched, idx_cols, dr_cols)
    return _CACHE[key]


_PREP_CACHE = {}


def _prep_cached(x, edge_index):
    key = (id(x), id(edge_index))
    if key not in _PREP_CACHE:
        _PREP_CACHE.clear()
        _PREP_CACHE[key] = _prep(x, edge_index)
    return _PREP_CACHE[key]


def run(x, edge_index, reps=1):
    ins, sched, idx_cols, dr_cols = _prep_cached(x, edge_index)
    nc = _get_nc(reps, sched, idx_cols, dr_cols)
    res = run_bass_kernel_spmd(nc, ins, core_ids=list(range(N_CORES)))
    parts = []
    for c in range(N_CORES):
        o = res.results[c]["out"]  # [128, 49, 32]
        parts.append(np.transpose(o, (1, 0, 2)).reshape(-1, D_FEAT))
    return np.concatenate(parts, axis=0)[:N_NODES]


def kernel(x, edge_index):
    return run(x, edge_index, reps=1)


# revision 7
# speedup vs baseline: 1.2096x; 1.0113x over previous
"""GNN message-passing (gather + segment_sum) Trainium2 Bass kernel.

Strategy (dst-sharded, SBUF-resident quartered feature table, zero
per-edge DMA):
  - Core c owns dst nodes [c*6272, (c+1)*6272) (49 groups of 128); host
    sorts edges by (core, dst-group, src-quarter) and packs int16 gather
    indices + per-slot group-relative dst values.
  - The x table lives in SBUF transposed ([feat, node]) with partition
    p = 32*lane + feat; lane i holds nodes [i*12544, (i+1)*12544), so
    the table is stored once (6.4MB, one linear DMA load - no per-edge
    descriptors, which cost ~1us each on HW and dominated the previous
    dma_gather kernel). Edges are routed to the lane owning their src.
  - Each 16-partition GpSimd core gathers feature planes for its lane's
    edge list via ap_gather (SBUF->SBUF ucode gather along the free dim,
    local node index < 12544 fits int16).
  - DVE StreamTranspose (32x32 blocks) of the [128, CALL] gather output
    directly yields [128 edge, 32 feat] chunks thanks to the 32i+f
    partition layout: chunk j = 32 edges per lane at positions
    [32j, 32j+32).
  - Per 128-edge chunk: one-hot B[edge, dst_rel] built on DVE (batched
    via broadcast APs), TensorE accumulates B^T @ msgs into a per-group
    PSUM partial, copied into an SBUF accumulator; one DMA writes the
    core's [128, 49, 32] output.

Self-contained: hardcodes the problem shapes from the spec.
"""

import numpy as np

import concourse.bass as bass
import concourse.tile as tile
from concourse import bacc, mybir
from concourse.alu_op_type import AluOpType
from concourse.bass_utils import run_bass_kernel_spmd

N_NODES = 50000
D_FEAT = 32
N_CORES = 8
G = 128  # dst nodes per group
GROUPS_PER_CORE = 49
QN = 12544  # nodes per lane quarter (4 * 12544 = 50176 >= 50000)
CALL = 2048  # ap_gather num_idxs per call (per 16-partition core)
CPC = CALL // 32  # chunks of 128 edges per call
BSEL_CH = 32  # chunks per one-hot build instruction


def _prep(x, edge_index):
    """Host-side packing. Returns per-core input maps + schedule constants."""
    src = np.asarray(edge_index[0], dtype=np.int64)
    dst = np.asarray(edge_index[1], dtype=np.int64)
    E = src.shape[0]

    ag = dst >> 7
    core = ag // GROUPS_PER_CORE
    g = ag % GROUPS_PER_CORE
    qtr = src // QN
    loc = src % QN

    okey = (core * GROUPS_PER_CORE + g) * 4 + qtr
    cnt = np.bincount(okey, minlength=N_CORES * GROUPS_PER_CORE * 4)
    cnt3 = cnt.reshape(N_CORES, GROUPS_PER_CORE, 4)
    # chunk count per group: max over cores and quarters so the SPMD program
    # is identical and every lane's 32-slot block fits.
    C = np.maximum(1, -(-cnt3.max(axis=(0, 2)) // 32))  # [49]
    chunks_tot = int(C.sum())
    n_calls = -(-chunks_tot // CPC)
    off = np.zeros(GROUPS_PER_CORE, np.int64)
    off[1:] = np.cumsum(C)[:-1]

    idx_cols = n_calls * (CALL // 16)
    dr_cols = n_calls * CPC

    order = np.argsort(okey, kind="stable")
    starts = np.concatenate([[0], np.cumsum(cnt)[:-1]])
    pos = np.arange(E) - starts[okey[order]]

    core_s = core[order]
    g_s = g[order]
    qtr_s = qtr[order]
    loc_s = loc[order]
    dst_s = dst[order]

    l = pos >> 5
    a = pos & 31
    k = off[g_s] + l
    m = k // CPC
    j = k % CPC
    t = 32 * j + a
    col = m * (CALL // 16) + (t >> 4)
    row = 32 * qtr_s + (t & 15)

    idx_arr = np.zeros((N_CORES, 128, idx_cols), np.int16)
    idxval = loc_s.astype(np.int16)
    idx_arr[core_s, row, col] = idxval
    idx_arr[core_s, row + 16, col] = idxval

    dr_arr = np.full((N_CORES, 128, dr_cols), -1.0, np.float32)
    q = 32 * qtr_s + a
    dr_arr[core_s, q, m * CPC + j] = (dst_s & 127).astype(np.float32)

    xT = np.zeros((D_FEAT, 4 * QN), np.float32)
    xT[:, :N_NODES] = np.asarray(x, dtype=np.float32).T
    iota = np.broadcast_to(
        np.arange(G, dtype=np.float32)[None, :], (128, G)
    ).copy()

    ins = []
    for c in range(N_CORES):
        ins.append(
            {
                "xT": xT,
                "idx16": idx_arr[c],
                "dstrel": dr_arr[c],
                "iota": iota,
            }
        )
    sched = (tuple(int(v) for v in C), n_calls)
    return ins, sched, idx_cols, dr_cols


def _build(reps, sched, idx_cols, dr_cols):
    C, n_calls = sched
    chunks_tot = sum(C)
    nc = bacc.Bacc(
        "TRN2", target_bir_lowering=False, debug=False, num_devices=N_CORES
    )
    f32 = mybir.dt.float32
    xT = nc.dram_tensor("xT", [D_FEAT, 4 * QN], f32, kind="ExternalInput")
    idx16 = nc.dram_tensor(
        "idx16", [128, idx_cols], mybir.dt.int16, kind="ExternalInput"
    )
    dstrel = nc.dram_tensor("dstrel", [128, dr_cols], f32, kind="ExternalInput")
    iota = nc.dram_tensor("iota", [128, G], f32, kind="ExternalInput")
    out = nc.dram_tensor(
        "out", [128, GROUPS_PER_CORE, D_FEAT], f32, kind="ExternalOutput"
    )

    # chunk -> (group, local chunk index)
    chunk_map = []
    for gg in range(GROUPS_PER_CORE):
        for ll in range(C[gg]):
            chunk_map.append((gg, ll))

    with tile.TileContext(nc) as tc:
        with (
            tc.tile_pool(name="meta", bufs=1) as meta,
            tc.tile_pool(name="tab", bufs=2) as tabp,
            tc.tile_pool(name="msgT", bufs=2) as msgTp,
            tc.tile_pool(name="msgt", bufs=2) as msgtp,
            tc.tile_pool(name="bsel", bufs=2) as bselp,
            tc.tile_pool(name="ps", bufs=4, space="PSUM") as psp,
            tc.tile_pool(name="osb", bufs=2) as osbp,
        ):
            idx_t = meta.tile([128, idx_cols], mybir.dt.int16)
            nc.sync.dma_start(idx_t[:], idx16.ap())
            dr_t = meta.tile([128, dr_cols], f32)
            nc.sync.dma_start(dr_t[:], dstrel.ap())
            iota_t = meta.tile([128, G], f32)
            nc.sync.dma_start(iota_t[:], iota.ap())

            def body(_=None):
                out_sb = osbp.tile([128, GROUPS_PER_CORE * D_FEAT], f32)
                tab = tabp.tile([128, QN], f32)
                for i in range(4):
                    nc.sync.dma_start(
                        tab[32 * i : 32 * i + 32, :],
                        xT.ap()[:, i * QN : (i + 1) * QN],
                    )
                ps_open = None
                for mm in range(n_calls):
                    msgs_T = msgTp.tile([128, CALL], f32)
                    nc.gpsimd.ap_gather(
                        msgs_T[:],
                        tab[:],
                        idx_t[:, mm * (CALL // 16) : (mm + 1) * (CALL // 16)],
                        128,
                        QN,
                        1,
                        CALL,
                    )
                    msgs_t = msgtp.tile([128, CALL], f32)
                    nc.vector.transpose(msgs_t[:], msgs_T[:])
                    for hb in range(CPC // BSEL_CH):
                        bt = bselp.tile([128, BSEL_CH, G], f32)
                        c0 = mm * CPC + hb * BSEL_CH
                        dr_b = (
                            dr_t[:, c0 : c0 + BSEL_CH]
                            .unsqueeze(2)
                            .broadcast_to((128, BSEL_CH, G))
                        )
                        iota_b = (
                            iota_t[:]
                            .unsqueeze(1)
                            .broadcast_to((128, BSEL_CH, G))
                        )
                        nc.vector.scalar_tensor_tensor(
                            bt[:],
                            iota_b,
                            0.0,
                            dr_b,
                            AluOpType.add,
                            AluOpType.is_equal,
                        )
                        for cc in range(BSEL_CH):
                            k = mm * CPC + hb * BSEL_CH + cc
                            if k >= chunks_tot:
                                continue
                            gg, ll = chunk_map[k]
                            if ll == 0:
                                ps_open = psp.tile([128, D_FEAT], f32)
                            e0 = 32 * (hb * BSEL_CH + cc)
                            nc.tensor.matmul(
                                out=ps_open[:],
                                lhsT=bt[:, cc, :],
                                rhs=msgs_t[:, e0 : e0 + D_FEAT],
                                start=(ll == 0),
                                stop=(ll == C[gg] - 1),
                            )
                            if ll == C[gg] - 1:
                                nc.scalar.copy(
                                    out_sb[:, gg * D_FEAT : (gg + 1) * D_FEAT],
                                    ps_open[:],
                                )
                nc.sync.dma_start(out.ap(), out_sb[:])

            if reps == 1:
                body()
            else:
                with tc.For_i(0, reps) as _i:
                    body(_i)
    nc.compile()
    return nc


_CACHE = {}


def _get_nc(reps, sched, idx_cols, dr_cols):
    key = (reps, sched, idx_cols, dr_cols)
    if key not in _CACHE:
        _CACHE[key] = _build(reps, sched, idx_cols, dr_cols)
    return _CACHE[key]


_PREP_CACHE = {}


def _prep_cached(x, edge_index):
    key = (id(x), id(edge_index))
    if key not in _PREP_CACHE:
        _PREP_CACHE.clear()
        _PREP_CACHE[key] = _prep(x, edge_index)
    return _PREP_CACHE[key]


def run(x, edge_index, reps=1):
    ins, sched, idx_cols, dr_cols = _prep_cached(x, edge_index)
    nc = _get_nc(reps, sched, idx_cols, dr_cols)
    res = run_bass_kernel_spmd(nc, ins, core_ids=list(range(N_CORES)))
    parts = []
    for c in range(N_CORES):
        o = res.results[c]["out"]  # [128, 49, 32]
        parts.append(np.transpose(o, (1, 0, 2)).reshape(-1, D_FEAT))
    return np.concatenate(parts, axis=0)[:N_NODES]


def kernel(x, edge_index):
    return run(x, edge_index, reps=1)
